# revision 1
# baseline (speedup 1.0000x reference)
"""Trainium2 Bass kernel for nn_MetaMultiHeadSelfAttention_45810121179385.

Multi-head causal self-attention: B=4, S=2048, D=1024, H=16 heads (hd=64).

Sharding (8 NeuronCores): batch (4) x head-group (2 groups of 8 heads).
Core c handles batch b = c//2, head group g = c%2:
  - QKV projections for its 512 head-dims (tensor parallel on d_k rows)
  - attention for its 8 heads (full sequence, causal)
  - partial o_proj (columns of o_proj = rows of o_proj^T for its 512 v-dims)
Host sums the two partial outputs per batch and stacks batches.

Device layouts (per core):
  xT   [1024, 2048]  x[b] transposed (d on partitions)
  wqT/wkT/wvT [1024, 512]  projection weights transposed (d_in on partitions)
  woT  [512, 1024]   o_proj columns for this group, transposed (v on partitions)
  yT   [1024, 2048]  partial output transposed [m, s]

All matmuls run as float32r (full fp32 precision at 1 cycle/row for N>=256).
Attention is computed in scores^T layout [k_pos, q] so no transposes are
needed anywhere: softmax denominators come from a ones-column appended to V
inside the P@V matmul, and the causal mask is one 128x128 triangular
multiply per diagonal tile.
"""

import functools
import os
import sys

import numpy as np

sys.path.insert(0, "/opt/trn_rl_repo")

import concourse.bass as bass  # noqa: E402
import concourse.tile as tile  # noqa: E402
from concourse import bacc, mybir  # noqa: E402
from concourse.bass_utils import run_bass_kernel_spmd  # noqa: E402

F32 = mybir.dt.float32
F32R = mybir.dt.float32r
EXP = mybir.ActivationFunctionType.Exp

B, S, D, H, HD = 4, 2048, 1024, 16, 64
NCORES = 8
HPC = 8          # heads per core
GD = HPC * HD    # 512 head-dims per core
NKT = S // 128   # 16 kpos tiles
NQC = S // 512   # 4 q chunks of 512
NDC = D // 128   # 8 contraction chunks for projections
NVT = GD // 128  # 4 dk/v chunks per core
SCALE = 1.0 / np.sqrt(HD)

# tuning knobs, overridable per-variant (see build_program(variant))
DEFAULT_OPTS = {
    "sc_piece": 1024,   # scores piece width (1024 or 2048)
    "sc_bufs": 2,       # scores psum double-buffering
    "pad256": 0,        # pad small score chunks to 256 (avoid 4-cyc f32r)
    "e_bufs": 4,
    "r_bufs": 4,
}
OPTS = dict(DEFAULT_OPTS)

VARIANTS = {
    "": {},
    "v1sc": {"sc_piece": 2048, "sc_bufs": 1},
    "pad": {"pad256": 1},
    "eb4": {"e_bufs": 4, "r_bufs": 4},
    "eb3": {"e_bufs": 3, "r_bufs": 3},
}


def _mha_tile_kernel(tc, xT, wqT, wkT, wvT, woT, mask, yT):
    nc = tc.nc

    with (
        tc.tile_pool(name="weights", bufs=1) as wpool,
        tc.tile_pool(name="consts", bufs=1) as cpool,
        tc.tile_pool(name="qkv", bufs=1) as qkv,
    ):
        # ---- resident SBUF tensors ----
        wo_sb = wpool.tile([128, NVT, D], F32R, tag="wo")
        mask_sb = cpool.tile([128, 128], F32R, tag="mask")

        qt_sb = qkv.tile([128, NVT, S], F32R, tag="qt")   # Q^T [dk, s]
        kt_sb = qkv.tile([128, NVT, S], F32R, tag="kt")   # K^T [dk, s]
        # V in [kpos, dv] layout, 65 cols per head (64 data + ones col)
        v_sb = qkv.tile([128, NKT, HPC, HD + 1], F32R, tag="v")

        # ================= Phase 1: QKV projections =================
        with (
            tc.tile_pool(name="wqkv", bufs=1) as wqkv,
            tc.tile_pool(name="x", bufs=2) as xpool,
            tc.tile_pool(name="ps_proj", bufs=6, space="PSUM") as ps_proj,
        ):
            wq_sb = wqkv.tile([128, NDC, GD], F32R, tag="wq")
            wk_sb = wqkv.tile([128, NDC, GD], F32R, tag="wk")
            wv_sb = wqkv.tile([128, NDC, GD], F32R, tag="wv")
            # issue order matters: first matmul needs wq + x[0] only — load
            # those (in halves) first so PE starts ~7us in, not ~33us
            wqT_r = wqT.rearrange("(k p) g -> p k g", p=128)
            for sc in range(NQC):
                x_sb = xpool.tile([128, NDC, 512], F32R, tag="x")
                xT_r = xT[:, 512 * sc : 512 * (sc + 1)].rearrange(
                    "(k p) s -> p k s", p=128
                )
                if sc == 0:
                    # fine-grained interleave so the first matmuls start ~3us in
                    for lo, hi in ((0, 2), (2, 4), (4, 8)):
                        nc.sync.dma_start(
                            out=wq_sb[:, lo:hi, :], in_=wqT_r[:, lo:hi, :]
                        )
                        nc.sync.dma_start(
                            out=x_sb[:, lo:hi, :], in_=xT_r[:, lo:hi, :]
                        )
                    nc.sync.dma_start(
                        out=wk_sb, in_=wkT.rearrange("(k p) g -> p k g", p=128)
                    )
                    nc.sync.dma_start(
                        out=wv_sb, in_=wvT.rearrange("(k p) g -> p k g", p=128)
                    )
                else:
                    h_nd = NDC // 2
                    nc.sync.dma_start(out=x_sb[:, 0:h_nd, :], in_=xT_r[:, 0:h_nd, :])
                    nc.sync.dma_start(out=x_sb[:, h_nd:, :], in_=xT_r[:, h_nd:, :])
                if sc == 1:
                    nc.sync.dma_start(out=mask_sb, in_=mask)
                    nc.sync.dma_start(
                        out=wo_sb, in_=woT.rearrange("(t p) m -> p t m", p=128)
                    )
                # ones columns for V tiles of this s-chunk
                for vt in range(NVT):
                    kti = 4 * sc + vt
                    nc.vector.tensor_copy(
                        out=v_sb[:, kti, :, HD : HD + 1],
                        in_=nc.const_aps.tensor(1.0, [128, HPC, 1], F32),
                    )
                for t in range(NVT):
                    ps_q = ps_proj.tile([128, 512], F32, tag="ps")
                    for k in range(NDC):
                        nc.tensor.matmul(
                            ps_q,
                            lhsT=wq_sb[:, k, 128 * t : 128 * (t + 1)],
                            rhs=x_sb[:, k, :],
                            start=(k == 0),
                            stop=(k == NDC - 1),
                        )
                    nc.vector.tensor_copy(
                        out=qt_sb[:, t, 512 * sc : 512 * (sc + 1)], in_=ps_q
                    )
                    ps_k = ps_proj.tile([128, 512], F32, tag="ps")
                    for k in range(NDC):
                        nc.tensor.matmul(
                            ps_k,
                            lhsT=wk_sb[:, k, 128 * t : 128 * (t + 1)],
                            rhs=x_sb[:, k, :],
                            start=(k == 0),
                            stop=(k == NDC - 1),
                        )
                    nc.vector.tensor_copy(
                        out=kt_sb[:, t, 512 * sc : 512 * (sc + 1)], in_=ps_k
                    )
                for vt in range(NVT):
                    kti = 4 * sc + vt
                    ps_v = ps_proj.tile([128, 512], F32, tag="ps")
                    for k in range(NDC):
                        nc.tensor.matmul(
                            ps_v,
                            lhsT=x_sb[:, k, 128 * vt : 128 * (vt + 1)],
                            rhs=wv_sb[:, k, :],
                            start=(k == 0),
                            stop=(k == NDC - 1),
                        )
                    # scatter into per-head 65-col groups (data cols 0..63)
                    nc.scalar.copy(
                        out=v_sb[:, kti, :, 0:HD],
                        in_=ps_v.rearrange("p (h d) -> p h d", h=HPC),
                    )

        # ================= Phase 2: attention per head =================
        with tc.tile_pool(name="outT", bufs=1) as opool:
            ot_sb = opool.tile([128, NVT, S], F32R, tag="ot")  # attn out^T [v, q]
            self_attention(tc, qt_sb, kt_sb, v_sb, ot_sb, mask_sb)

            # ================= Phase 3: output projection =================
            with (
                tc.tile_pool(name="ysb", bufs=2) as ypool,
                tc.tile_pool(name="ps_o", bufs=6, space="PSUM") as ps_o,
            ):
                for m in range(D // 128):
                    y_sb = ypool.tile([128, S], F32, tag="y")
                    for qc in range(NQC):
                        ps_y = ps_o.tile([128, 512], F32, tag="psy")
                        for t in range(NVT):
                            nc.tensor.matmul(
                                ps_y,
                                lhsT=wo_sb[:, t, 128 * m : 128 * (m + 1)],
                                rhs=ot_sb[:, t, 512 * qc : 512 * (qc + 1)],
                                start=(t == 0),
                                stop=(t == NVT - 1),
                            )
                        nc.scalar.copy(
                            out=y_sb[:, 512 * qc : 512 * (qc + 1)], in_=ps_y
                        )
                        if qc % 2 == 1:  # stream out in halves
                            nc.sync.dma_start(
                                out=yT[128 * m : 128 * (m + 1), 1024 * (qc // 2) : 1024 * (qc // 2 + 1)],
                                in_=y_sb[:, 1024 * (qc // 2) : 1024 * (qc // 2 + 1)],
                            )


def self_attention(tc, qt_sb, kt_sb, v_sb, ot_sb, mask_sb):
    nc = tc.nc
    # DRAM bounce buffer for broadcasting softmax reciprocal rows across
    # partitions (SBUF APs cannot have zero-step partition dims; DRAM can).
    rb_dram = nc.dram_tensor("rb_dram", [HPC, NQC, 512], F32).ap()
    PIECE = OPTS["sc_piece"]
    with (
        tc.tile_pool(name="expT", bufs=OPTS["e_bufs"]) as epool,
        tc.tile_pool(name="rtiles", bufs=OPTS["r_bufs"]) as rpool,
        tc.tile_pool(name="ps_sc", bufs=OPTS["sc_bufs"], space="PSUM") as ps_sc,
        tc.tile_pool(name="ps_pv", bufs=4, space="PSUM") as ps_pv,
    ):
        for h in range(HPC):
            t_h = h // 2
            p_h = 64 * (h % 2)
            pv = [
                ps_pv.tile([65, 512], F32, tag="pv", name=f"pv{h}_{i}")
                for i in range(NQC)
            ]
            for kt in range(NKT):
                c_lo = 128 * kt
                W = S - c_lo
                e_t = epool.tile([128, S], F32R, tag="e")
                # scores^T[k, q] for k-tile kt, q in [c_lo, S), in <=1024-wide
                # pieces (2 PSUM banks each, double-buffered) so TensorE can
                # run piece N+1's matmuls while ScalarE exps piece N.
                pieces = [(c_lo, min(PIECE, W))]
                if W > PIECE:
                    pieces.append((c_lo + PIECE, W - PIECE))
                for a, w in pieces:
                    sc_ps = ps_sc.tile([128, PIECE], F32, tag="sc")
                    col = 0
                    while col < w:
                        ncols = min(512 - (col % 512), w - col)
                        if (
                            OPTS["pad256"]
                            and ncols < 256
                            and col % 512 == 0
                            and col + 256 <= PIECE
                            and a + col + 256 <= S
                        ):
                            ncols = 256
                        nc.tensor.matmul(
                            sc_ps[:, col : col + ncols],
                            lhsT=kt_sb[p_h : p_h + 64, t_h, c_lo : c_lo + 128],
                            rhs=qt_sb[p_h : p_h + 64, t_h, a + col : a + col + ncols],
                            start=True,
                            stop=True,
                        )
                        col += ncols
                    # exp(scores/8)
                    nc.scalar.activation(
                        out=e_t[:, a : a + w],
                        in_=sc_ps[:, 0:w],
                        func=EXP,
                        scale=SCALE,
                    )
                # causal mask on the diagonal 128x128 block
                nc.vector.tensor_mul(
                    e_t[:, c_lo : c_lo + 128],
                    e_t[:, c_lo : c_lo + 128],
                    mask_sb,
                )
                # PV accumulation (+ denominator via ones column)
                for qc in range(kt // 4, NQC):
                    q0 = 512 * qc
                    c0 = max(q0, c_lo)
                    nc.tensor.matmul(
                        pv[qc][:, c0 - q0 : 512],
                        lhsT=v_sb[:, kt, h, :],
                        rhs=e_t[:, c0 : q0 + 512],
                        start=(kt == 0),
                        stop=(kt == 4 * qc + 3),
                    )
                # normalize each q-chunk as soon as its accumulation ends
                if kt % 4 == 3:
                    qc = kt // 4
                    q0 = 512 * qc
                    r_t = rpool.tile([65, 512], F32, tag="r")
                    nc.vector.reciprocal(
                        out=r_t[64:65, :], in_=pv[qc][64:65, :]
                    )
                    nc.sync.dma_start(out=rb_dram[h, qc, :], in_=r_t[64:65, :])
                    rb_t = rpool.tile([64, 512], F32, tag="rb")
                    src = rb_dram[h, qc, :]
                    nc.sync.dma_start(
                        out=rb_t,
                        in_=bass.AP(
                            tensor=src.tensor,
                            offset=src.offset,
                            ap=[[0, 64]] + list(src.ap),
                        ),
                    )
                    if p_h == 0:
                        nc.vector.tensor_mul(
                            ot_sb[0:64, t_h, q0 : q0 + 512],
                            pv[qc][0:64, :],
                            rb_t,
                        )
                    else:
                        st_t = rpool.tile([64, 512], F32R, tag="st")
                        nc.vector.tensor_mul(st_t, pv[qc][0:64, :], rb_t)
                        nc.sync.dma_start(
                            out=ot_sb[64:128, t_h, q0 : q0 + 512], in_=st_t
                        )

@functools.lru_cache(maxsize=8)
def build_program(variant=None):
    if variant is None:
        variant = os.environ.get("MHA_VARIANT", "")
    OPTS.clear()
    OPTS.update(DEFAULT_OPTS)
    OPTS.update(VARIANTS[variant])
    nc = bacc.Bacc("TRN2", target_bir_lowering=False, debug=False)
    xT = nc.dram_tensor("xT", [D, S], F32R, kind="ExternalInput").ap()
    wqT = nc.dram_tensor("wqT", [D, GD], F32R, kind="ExternalInput").ap()
    wkT = nc.dram_tensor("wkT", [D, GD], F32R, kind="ExternalInput").ap()
    wvT = nc.dram_tensor("wvT", [D, GD], F32R, kind="ExternalInput").ap()
    woT = nc.dram_tensor("woT", [GD, D], F32R, kind="ExternalInput").ap()
    mask = nc.dram_tensor("mask", [128, 128], F32R, kind="ExternalInput").ap()
    yT = nc.dram_tensor("yT", [D, S], F32, kind="ExternalOutput").ap()
    with tile.TileContext(nc) as tc:
        _mha_tile_kernel(tc, xT, wqT, wkT, wvT, woT, mask, yT)
    nc.compile()
    return nc


def make_in_maps(x, q_proj, k_proj, v_proj, o_proj):
    x = np.ascontiguousarray(x, dtype=np.float32)
    mask = np.triu(np.ones((128, 128), dtype=np.float32))  # keep iff col >= row
    in_maps = []
    for c in range(NCORES):
        b, g = divmod(c, 2)
        sl = slice(GD * g, GD * (g + 1))
        in_maps.append(
            {
                "xT": np.ascontiguousarray(x[b].T),
                "wqT": np.ascontiguousarray(np.asarray(q_proj)[sl, :].T),
                "wkT": np.ascontiguousarray(np.asarray(k_proj)[sl, :].T),
                "wvT": np.ascontiguousarray(np.asarray(v_proj)[sl, :].T),
                "woT": np.ascontiguousarray(np.asarray(o_proj)[:, sl].T),
                "mask": mask,
            }
        )
    return in_maps


def gather_output(results):
    outs = [np.asarray(r["yT"], dtype=np.float32) for r in results]
    return np.stack(
        [(outs[2 * b] + outs[2 * b + 1]).T for b in range(B)], axis=0
    )


def kernel(x, q_proj, k_proj, v_proj, o_proj, _trace=False, _trace_kwargs=None):
    nc = build_program()
    in_maps = make_in_maps(x, q_proj, k_proj, v_proj, o_proj)
    res = run_bass_kernel_spmd(
        nc,
        in_maps,
        core_ids=list(range(NCORES)),
        trace=_trace,
        **(_trace_kwargs or {}),
    )
    y = gather_output(res.results)
    if _trace:
        kernel.last_result = res
    return y

